# revision 8
# baseline (speedup 1.0000x reference)
"""Gaussian KDE (bandwidth=0.5) on 8 TRN2 NeuronCores.

out[j] = sum_i mask_i * exp(-|s_i - l_j|^2 / bw^2), normalized to sum 1.

Strategy (data-parallel over samples):
  - core c gets samples[c*2048:(c+1)*2048] and all 8192 locations.
  - exp argument is expanded as a K=3 matmul:
        arg[p,i] = 8*(lx_j*sx_i + ly_j*sy_i) + t_i + bias_j
    with stationary lhsT = [lx; ly; 1] (per 128-location block),
    moving rhs = [8*sx; 8*sy; t],  t_i = -4*|s_i|^2 + 500*(inx_i+iny_i),
    bias_j = -4*|l_j|^2 - 1000  (ACT per-partition bias).
    For in-bbox samples (inx+iny==2) this is exactly -4*|s-l|^2; otherwise
    it is <= -500 and exp underflows to exactly 0 (torch mask semantics).
  - ScalarE ACT computes exp over each [128, 2048] PSUM tile with a fused
    free-axis accumulate (accum_out) -> per-core partial sums [128, 64].
  - AllReduce over the 8 cores, then each core normalizes on-device.

Location index mapping: j = p*64 + b (partition p, block b), so the final
[128, 64] SBUF accumulator stores row-major j and the output DMA is
contiguous.
"""

import sys

sys.path.insert(0, "/opt/trn_rl_repo")

import numpy as np

N_CORES = 8
NS = 16384
NL = 8192
NS_SH = NS // N_CORES  # 2048 samples per core
NBLK = NL // 128  # 64 location blocks
MM_N = 512  # fp32 moving-operand limit
BW = 0.5
INV_BW2 = 1.0 / (BW * BW)  # 4.0
C2 = 2.0 * INV_BW2  # 8.0
PEN = 500.0
FOLD = 2.0 * PEN

_STATE = {}


def build_nc():
    import concourse.bacc as bacc
    import concourse.mybir as mybir
    import concourse.tile as tile

    f32 = mybir.dt.float32
    AX = mybir.AxisListType
    AF = mybir.ActivationFunctionType
    AL = mybir.AluOpType

    nc = bacc.Bacc(None, target_bir_lowering=False, num_devices=N_CORES)

    s_t = nc.declare_dram_parameter("samples_t", [2, NS_SH], f32, isOutput=False)
    l_t = nc.declare_dram_parameter("locations_t", [2, NL], f32, isOutput=False)
    l_n = nc.declare_dram_parameter("locations_n", [128, 2 * NBLK], f32, isOutput=False)
    out_d = nc.declare_dram_parameter("out", [128, NBLK], f32, isOutput=True)

    with tile.TileContext(nc) as tc:
        with tc.tile_pool(name="const", bufs=1) as cpool, \
             tc.tile_pool(name="dram", bufs=1, space="DRAM") as dpool, \
             tc.tile_pool(name="escr", bufs=2) as epool, \
             tc.tile_pool(name="ps", bufs=2, space="PSUM") as ppool:

            LT = cpool.tile([3, NL], f32)  # stationary [1; lx; ly], block-major
            LL = cpool.tile([128, 2 * NBLK], f32)  # [lx | ly] natural
            S2 = cpool.tile([2, NS_SH], f32)  # [sx; sy]
            R = cpool.tile([3, NS_SH], f32)  # moving [t; 8sx; 8sy]
            R8 = cpool.tile([2, NS_SH], f32)  # staging for 8*S2 (base partition 0)
            ones2 = cpool.tile([2, 1], f32)  # K=2 fold weights
            mt = cpool.tile([1, 2], f32)  # (mx, my) at partition 0
            B = cpool.tile([128, NBLK], f32)  # ACT bias
            acc = cpool.tile([128, NBLK], f32)  # partial kernel sums
            m2 = cpool.tile([2, 1], f32)  # (mx, my) bbox bounds
            sq = cpool.tile([2, NS_SH], f32)
            A2 = cpool.tile([2, NS_SH], f32)
            U = cpool.tile([2, NS_SH], f32)
            rm = cpool.tile([128, 2], f32)
            t1 = cpool.tile([128, NBLK], f32)
            t2 = cpool.tile([128, NBLK], f32)
            G = cpool.tile([128, NBLK], f32)
            Gs = cpool.tile([128, 1], f32)
            tot = cpool.tile([1, 1], f32)
            rtot = cpool.tile([1, 1], f32)
            rb = cpool.tile([128, 1], f32)
            ones1 = cpool.tile([1, 128], f32)

            partial = dpool.tile([128, NBLK], f32)
            allsum = dpool.tile([128, NBLK], f32, addr_space="Shared")

            # ---- input loads (all contiguous) ----
            nc.sync.dma_start(out=LT[1:3, :], in_=l_t[:, :])
            nc.vector.memset(LT[0:1, :], 1.0)
            nc.sync.dma_start(out=LL[:, :], in_=l_n[:, :])
            nc.sync.dma_start(out=S2[:, :], in_=s_t[:, :])

            lx = LL[:, 0:NBLK]
            ly = LL[:, NBLK : 2 * NBLK]

            # ---- location-side prep: bias and bbox bounds ----
            nc.vector.tensor_tensor(t1[:], lx, lx, AL.mult)
            nc.vector.tensor_tensor(t2[:], ly, ly, AL.mult)
            nc.vector.tensor_tensor(t1[:], t1[:], t2[:], AL.add)
            nc.vector.tensor_scalar(B[:], t1[:], -INV_BW2, -FOLD, AL.mult, AL.add)

            nc.vector.tensor_reduce(
                rm[:, 0:1], lx, axis=AX.X, op=AL.max, apply_absolute_value=True
            )
            nc.vector.tensor_reduce(
                rm[:, 1:2], ly, axis=AX.X, op=AL.max, apply_absolute_value=True
            )
            nc.gpsimd.tensor_reduce(mt[:, :], rm[:, :], axis=AX.C, op=AL.max)
            # scatter (mx, my) to partitions 0 and 1 (DMA has no base restriction)
            nc.sync.dma_start(out=m2[0:1, :], in_=mt[:, 0:1])
            nc.sync.dma_start(out=m2[1:2, :], in_=mt[:, 1:2])

            # ---- sample-side prep (all compute at base partition 0) ----
            nc.vector.tensor_scalar(R8[:], S2[:], C2, None, AL.mult)
            nc.sync.dma_start(out=R[1:3, :], in_=R8[:])
            nc.vector.tensor_tensor(sq[:], S2[:], S2[:], AL.mult)
            nc.scalar.activation(A2[:], S2[:], AF.Abs)
            # U = -4*s^2 + 500*(|s| < m)
            nc.vector.tensor_scalar(U[:], sq[:], -INV_BW2, None, AL.mult)
            nc.vector.tensor_scalar(sq[:], A2[:], m2[:, 0:1], PEN, AL.is_lt, AL.mult)
            nc.vector.tensor_tensor(U[:], U[:], sq[:], AL.add)
            # fold the two coordinate rows via PE: t = U[0] + U[1]
            nc.vector.memset(ones2[:], 1.0)
            psF = ppool.tile([1, NS_SH], f32, tag="ps")
            for n in range(NS_SH // MM_N):
                nc.tensor.matmul(
                    psF[:, n * MM_N : (n + 1) * MM_N],
                    lhsT=ones2[:],
                    rhs=U[:, n * MM_N : (n + 1) * MM_N],
                    start=True,
                    stop=True,
                )
            nc.scalar.copy(R[0:1, :], psF[:])

            # ---- main loop: 64 location blocks ----
            for b in range(NBLK):
                ps = ppool.tile([128, NS_SH], f32, tag="ps")
                for n in range(NS_SH // MM_N):
                    nc.tensor.matmul(
                        ps[:, n * MM_N : (n + 1) * MM_N],
                        lhsT=LT[:, b * 128 : (b + 1) * 128],
                        rhs=R[:, n * MM_N : (n + 1) * MM_N],
                        start=True,
                        stop=True,
                    )
                es = epool.tile([128, NS_SH], f32, tag="es")
                nc.scalar.activation(
                    es[:],
                    ps[:],
                    AF.Exp,
                    bias=B[:, b : b + 1],
                    scale=1.0,
                    accum_out=acc[:, b : b + 1],
                )

            # ---- all-reduce partial sums, normalize on-device ----
            nc.sync.dma_start(out=partial[:, :], in_=acc[:])
            nc.gpsimd.collective_compute(
                "AllReduce",
                AL.add,
                replica_groups=[list(range(N_CORES))],
                ins=[partial[:, :]],
                outs=[allsum[:, :]],
            )
            nc.sync.dma_start(out=G[:], in_=allsum[:, :])
            nc.vector.tensor_reduce(Gs[:], G[:], axis=AX.X, op=AL.add)
            nc.gpsimd.tensor_reduce(tot[:], Gs[:], axis=AX.C, op=AL.add)
            nc.vector.reciprocal(rtot[:], tot[:])
            # broadcast 1/norm to all 128 partitions via PE (ones is LT row 2)
            psb = ppool.tile([128, 1], f32, tag="ps")
            nc.vector.memset(ones1[:], 1.0)
            nc.tensor.matmul(
                psb[:], lhsT=ones1[:], rhs=rtot[:], start=True, stop=True
            )
            nc.scalar.copy(rb[:], psb[:])
            nc.vector.tensor_scalar(G[:], G[:], rb[:], None, AL.mult)
            nc.sync.dma_start(out=out_d[:, :], in_=G[:])

    nc.compile()  # Bacc register allocation / DCE — required before walrus
    return nc


def _loc_layouts(locations):
    # locations_t: [2, 8192], column b*128+p holds location j = p*64+b
    lt = np.ascontiguousarray(
        locations.T.reshape(2, 128, NBLK).transpose(0, 2, 1).reshape(2, NL)
    )
    # locations_n: [128, 128], cols 0..63 = lx, 64..127 = ly, row p / col b = j=p*64+b
    ln3 = locations.reshape(128, NBLK, 2)
    ln = np.ascontiguousarray(
        np.concatenate([ln3[:, :, 0], ln3[:, :, 1]], axis=1)
    )
    return lt, ln


def make_in_maps(samples, locations):
    lt, ln = _loc_layouts(locations)
    in_maps = []
    for c in range(N_CORES):
        shard = samples[c * NS_SH : (c + 1) * NS_SH]
        in_maps.append(
            {
                "samples_t": np.ascontiguousarray(shard.T),
                "locations_t": lt,
                "locations_n": ln,
            }
        )
    return in_maps


def kernel(samples, locations):
    samples = np.ascontiguousarray(np.asarray(samples, dtype=np.float32))
    locations = np.ascontiguousarray(np.asarray(locations, dtype=np.float32))
    assert samples.shape == (NS, 2) and locations.shape == (NL, 2)

    from concourse.bass_utils import run_bass_kernel_spmd

    if "nc" not in _STATE:
        _STATE["nc"] = build_nc()
    nc = _STATE["nc"]

    in_maps = make_in_maps(samples, locations)
    res = run_bass_kernel_spmd(
        nc,
        in_maps,
        list(range(N_CORES)),
        trace=bool(_STATE.get("trace", False)),
    )
    _STATE["exec_time_ns"] = res.exec_time_ns
    _STATE["profile_json"] = res.profile_json
    return np.asarray(res.results[0]["out"], dtype=np.float32).reshape(NL)


# revision 19
# speedup vs baseline: 2.8702x; 2.8702x over previous
"""Gaussian KDE (bandwidth=0.5) on 8 TRN2 NeuronCores.

out[j] = sum_i mask_i * exp(-|s_i - l_j|^2 / bw^2), normalized to sum 1.

Strategy (data-parallel over samples):
  - core c gets samples[c*2048:(c+1)*2048] and all 8192 locations.
  - exp argument is expanded as a K=3 matmul:
        arg[p,i] = 8*(lx_j*sx_i + ly_j*sy_i) + t_i + bias_j
    with stationary lhsT = [lx; ly; 1] (per 128-location block),
    moving rhs = [8*sx; 8*sy; t],  t_i = -4*|s_i|^2 + 500*(inx_i+iny_i),
    bias_j = -4*|l_j|^2 - 1000  (ACT per-partition bias).
    For in-bbox samples (inx+iny==2) this is exactly -4*|s-l|^2; otherwise
    it is <= -500 and exp underflows to exactly 0 (torch mask semantics).
  - ScalarE ACT computes exp over each [128, 2048] PSUM tile with a fused
    free-axis accumulate (accum_out) -> per-core partial sums [128, 64].
  - AllReduce over the 8 cores, then each core normalizes on-device.

Location index mapping: j = p*64 + b (partition p, block b), so the final
[128, 64] SBUF accumulator stores row-major j and the output DMA is
contiguous.
"""

import sys

sys.path.insert(0, "/opt/trn_rl_repo")

import numpy as np

N_CORES = 8
NS = 16384
NL = 8192
NS_SH = NS // N_CORES  # 2048 samples per core
NBLK = NL // 128  # 64 location blocks
MM_N = 512  # fp32 moving-operand limit
BW = 0.5
INV_BW2 = 1.0 / (BW * BW)  # 4.0
C2 = 2.0 * INV_BW2  # 8.0
PEN = 500.0
FOLD = 2.0 * PEN
N_CHUNKS = 4  # all-reduce chunks overlapped with compute

_STATE = {}


def build_nc():
    import concourse.bacc as bacc
    import concourse.mybir as mybir
    import concourse.tile as tile

    f32 = mybir.dt.float32
    AX = mybir.AxisListType
    AF = mybir.ActivationFunctionType
    AL = mybir.AluOpType

    nc = bacc.Bacc(None, target_bir_lowering=False, num_devices=N_CORES)

    bf16 = mybir.dt.bfloat16
    s_t = nc.declare_dram_parameter("samples_t", [2, NS_SH], f32, isOutput=False)
    l_s = nc.declare_dram_parameter("loc_split", [6, NL], bf16, isOutput=False)
    l_n = nc.declare_dram_parameter("locations_n", [128, 2 * NBLK], f32, isOutput=False)
    out_d = nc.declare_dram_parameter("out", [128, NBLK], f32, isOutput=True)

    with tile.TileContext(nc) as tc:
        with tc.tile_pool(name="const", bufs=1) as cpool, \
             tc.tile_pool(name="dram", bufs=1, space="DRAM") as dpool, \
             tc.tile_pool(name="escr", bufs=2) as epool, \
             tc.tile_pool(name="ps", bufs=2, space="PSUM") as ppool:

            bf = bf16
            Lb = cpool.tile([9, NL], bf)  # stationary [1;1;1; lxh;lyh; lxh;lyh; lxl;lyl]
            LL = cpool.tile([128, 2 * NBLK], f32)  # [lx | ly] natural
            S2 = cpool.tile([2, NS_SH], f32)  # [sx; sy]
            Rb = cpool.tile([9, NS_SH], bf)  # moving [pen;th;tl; xh;yh; xl;yl; xh;yh]
            R8 = cpool.tile([2, NS_SH], f32)  # 8*S2 (base partition 0)
            hi2 = cpool.tile([2, NS_SH], bf)
            lo2 = cpool.tile([2, NS_SH], bf)
            tf = cpool.tile([1, NS_SH], f32)
            thb = cpool.tile([1, NS_SH], bf)
            tlb = cpool.tile([1, NS_SH], bf)
            penb = cpool.tile([1, NS_SH], bf)
            ones2 = cpool.tile([2, 1], f32)  # K=2 fold weights
            mt = cpool.tile([1, 2], f32)  # (mx, my) at partition 0
            B = cpool.tile([128, NBLK], f32)  # ACT bias
            acc = cpool.tile([128, NBLK], f32)  # partial kernel sums
            m2 = cpool.tile([2, 1], f32)  # (mx, my) bbox bounds
            sq = cpool.tile([2, NS_SH], f32)
            A2 = cpool.tile([2, NS_SH], f32)
            U = cpool.tile([2, NS_SH], f32)
            rm = cpool.tile([128, 2], f32)
            t1 = cpool.tile([128, NBLK], f32)
            t2 = cpool.tile([128, NBLK], f32)
            G = cpool.tile([128, NBLK], f32)
            Gs = cpool.tile([128, 1], f32)
            tot = cpool.tile([1, 1], f32)
            rtot = cpool.tile([1, 1], f32)
            rb = cpool.tile([128, 1], f32)
            ones1 = cpool.tile([1, 128], f32)

            GRP_W = NBLK // N_CHUNKS
            partials = [
                dpool.tile([128, GRP_W], f32, name=f"partial{g}")
                for g in range(N_CHUNKS)
            ]
            allsums = [
                dpool.tile([128, GRP_W], f32, addr_space="Shared", name=f"allsum{g}")
                for g in range(N_CHUNKS)
            ]

            # ---- input loads (all contiguous) ----
            nc.vector.memset(Lb[0:3, :], 1.0)
            nc.sync.dma_start(out=Lb[3:9, :], in_=l_s[:, :])
            nc.sync.dma_start(out=LL[:, :], in_=l_n[:, :])
            nc.sync.dma_start(out=S2[:, :], in_=s_t[:, :])

            lx = LL[:, 0:NBLK]
            ly = LL[:, NBLK : 2 * NBLK]

            # ---- location-side prep: bias and bbox bounds ----
            nc.vector.tensor_tensor(t1[:], lx, lx, AL.mult)
            nc.vector.tensor_tensor(t2[:], ly, ly, AL.mult)
            nc.vector.tensor_tensor(t1[:], t1[:], t2[:], AL.add)
            nc.vector.tensor_scalar(B[:], t1[:], -INV_BW2, None, AL.mult)

            nc.vector.tensor_reduce(
                rm[:, 0:1], lx, axis=AX.X, op=AL.max, apply_absolute_value=True
            )
            nc.vector.tensor_reduce(
                rm[:, 1:2], ly, axis=AX.X, op=AL.max, apply_absolute_value=True
            )
            nc.gpsimd.tensor_reduce(mt[:, :], rm[:, :], axis=AX.C, op=AL.max)
            # scatter (mx, my) to partitions 0 and 1 (DMA has no base restriction)
            nc.sync.dma_start(out=m2[0:1, :], in_=mt[:, 0:1])
            nc.sync.dma_start(out=m2[1:2, :], in_=mt[:, 1:2])

            # ---- sample-side prep (all compute at base partition 0) ----
            # hi/lo bf16 split of 8*s so the matmul can run in bf16 while
            # keeping ~f32 accuracy (hi*hi, hi*lo, lo*hi products, f32 PSUM).
            nc.vector.tensor_scalar(R8[:], S2[:], C2, None, AL.mult)
            nc.vector.tensor_copy(hi2[:], R8[:])
            nc.vector.tensor_tensor(lo2[:], R8[:], hi2[:], AL.subtract)
            nc.vector.tensor_tensor(sq[:], S2[:], S2[:], AL.mult)
            nc.scalar.activation(A2[:], S2[:], AF.Abs)
            # U = (|s| < m) in {0,1}
            nc.vector.tensor_scalar(U[:], A2[:], m2[:, 0:1], None, AL.is_lt)
            # PE folds across the two coordinate rows
            nc.vector.memset(ones2[:], 1.0)
            psS = ppool.tile([1, NS_SH], f32, tag="ps")
            for n in range(NS_SH // MM_N):
                nc.tensor.matmul(
                    psS[:, n * MM_N : (n + 1) * MM_N],
                    lhsT=ones2[:],
                    rhs=sq[:, n * MM_N : (n + 1) * MM_N],
                    start=True,
                    stop=True,
                )
            psC = ppool.tile([1, NS_SH], f32, tag="ps")
            for n in range(NS_SH // MM_N):
                nc.tensor.matmul(
                    psC[:, n * MM_N : (n + 1) * MM_N],
                    lhsT=ones2[:],
                    rhs=U[:, n * MM_N : (n + 1) * MM_N],
                    start=True,
                    stop=True,
                )
            # t = -4*|s|^2 split into th+tl (bf16 pair); pen = 500*(cx+cy)-1000
            nc.scalar.mul(tf[:], psS[:], -INV_BW2)
            nc.vector.tensor_copy(thb[:], tf[:])
            nc.vector.tensor_tensor(tlb[:], tf[:], thb[:], AL.subtract)
            nc.scalar.activation(
                penb[:], psC[:], AF.Copy, bias=-2.0 * PEN, scale=PEN
            )
            # assemble moving operand (DMA may write any base partition)
            nc.sync.dma_start(out=Rb[0:1, :], in_=penb[:])
            nc.sync.dma_start(out=Rb[1:2, :], in_=thb[:])
            nc.sync.dma_start(out=Rb[2:3, :], in_=tlb[:])
            nc.sync.dma_start(out=Rb[3:5, :], in_=hi2[:])
            nc.sync.dma_start(out=Rb[5:7, :], in_=lo2[:])
            nc.sync.dma_start(out=Rb[7:9, :], in_=hi2[:])

            # ---- main loop: 64 location blocks, chunked all-reduce overlap ----
            GRP = NBLK // N_CHUNKS
            for b in range(NBLK):
                ps = ppool.tile([128, NS_SH], f32, tag="ps")
                for n in range(NS_SH // MM_N):
                    nc.tensor.matmul(
                        ps[:, n * MM_N : (n + 1) * MM_N],
                        lhsT=Lb[:, b * 128 : (b + 1) * 128],
                        rhs=Rb[:, n * MM_N : (n + 1) * MM_N],
                        start=True,
                        stop=True,
                    )
                es = epool.tile([128, NS_SH], f32, tag="es")
                nc.scalar.activation(
                    es[:],
                    ps[:],
                    AF.Exp,
                    bias=B[:, b : b + 1],
                    scale=1.0,
                    accum_out=acc[:, b : b + 1],
                )
                if b % GRP == GRP - 1:
                    g = b // GRP
                    lo, hi = g * GRP, (g + 1) * GRP
                    nc.sync.dma_start(
                        out=partials[g][:, :], in_=acc[:, lo:hi]
                    )
                    nc.gpsimd.collective_compute(
                        "AllReduce",
                        AL.add,
                        replica_groups=[list(range(N_CORES))],
                        ins=[partials[g][:, :]],
                        outs=[allsums[g][:, :]],
                    )

            # ---- normalize on-device ----
            for g in range(N_CHUNKS):
                nc.sync.dma_start(
                    out=G[:, g * GRP : (g + 1) * GRP], in_=allsums[g][:, :]
                )
            nc.vector.tensor_reduce(Gs[:], G[:], axis=AX.X, op=AL.add)
            nc.gpsimd.tensor_reduce(tot[:], Gs[:], axis=AX.C, op=AL.add)
            nc.vector.reciprocal(rtot[:], tot[:])
            # broadcast 1/norm to all 128 partitions via PE (ones is LT row 2)
            psb = ppool.tile([128, 1], f32, tag="ps")
            nc.vector.memset(ones1[:], 1.0)
            nc.tensor.matmul(
                psb[:], lhsT=ones1[:], rhs=rtot[:], start=True, stop=True
            )
            nc.scalar.copy(rb[:], psb[:])
            nc.vector.tensor_scalar(G[:], G[:], rb[:], None, AL.mult)
            nc.sync.dma_start(out=out_d[:, :], in_=G[:])

    nc.compile()  # Bacc register allocation / DCE — required before walrus
    return nc


def _loc_layouts(locations):
    from ml_dtypes import bfloat16

    # block-permuted transpose: column b*128+p holds location j = p*64+b
    lt = np.ascontiguousarray(
        locations.T.reshape(2, 128, NBLK).transpose(0, 2, 1).reshape(2, NL)
    )
    # hi/lo bf16 split (lossless re-encoding of the f32 coords; rows are
    # [lxh, lyh, lxh, lyh, lxl, lyl] matching the K=9 stationary layout)
    lth = lt.astype(bfloat16)
    ltl = (lt - lth.astype(np.float32)).astype(bfloat16)
    ls = np.ascontiguousarray(np.concatenate([lth, lth, ltl], axis=0))
    # locations_n: [128, 128], cols 0..63 = lx, 64..127 = ly, row p / col b = j=p*64+b
    ln3 = locations.reshape(128, NBLK, 2)
    ln = np.ascontiguousarray(
        np.concatenate([ln3[:, :, 0], ln3[:, :, 1]], axis=1)
    )
    return ls, ln


def make_in_maps(samples, locations):
    ls, ln = _loc_layouts(locations)
    in_maps = []
    for c in range(N_CORES):
        shard = samples[c * NS_SH : (c + 1) * NS_SH]
        in_maps.append(
            {
                "samples_t": np.ascontiguousarray(shard.T),
                "loc_split": ls,
                "locations_n": ln,
            }
        )
    return in_maps


def kernel(samples, locations):
    samples = np.ascontiguousarray(np.asarray(samples, dtype=np.float32))
    locations = np.ascontiguousarray(np.asarray(locations, dtype=np.float32))
    assert samples.shape == (NS, 2) and locations.shape == (NL, 2)

    from concourse.bass_utils import run_bass_kernel_spmd

    if "nc" not in _STATE:
        _STATE["nc"] = build_nc()
    nc = _STATE["nc"]

    in_maps = make_in_maps(samples, locations)
    res = run_bass_kernel_spmd(
        nc,
        in_maps,
        list(range(N_CORES)),
        trace=bool(_STATE.get("trace", False)),
    )
    _STATE["exec_time_ns"] = res.exec_time_ns
    _STATE["profile_json"] = res.profile_json
    return np.asarray(res.results[0]["out"], dtype=np.float32).reshape(NL)
